# revision 39
# baseline (speedup 1.0000x reference)
"""Bipartite graph convolution (GCMC-style) Trainium2 kernel, 8-core SPMD.

Math (reference): per-rating masks M_r = (adj == r), r=1..5,
  out_u = relu(d_u * sum_r (M_r @ v_feat) @ W_u[r]),  d_u = 1/deg_u
  out_v = relu(d_v * sum_r (M_r.T @ u_feat) @ W_v[r]), d_v = 1/deg_v

Device formulation (per core, u-rows sharded 1024/core), v4:
  Fold weights on host: P_b = v_feat @ W~_u[b]  [8192, 64] per basis b,
  Q_b = u_shard @ W~_v[b] [1024, 64], basis {adj, M2, M3, ramp4, M5}
  (adj = sum_r r*M_r) so only 4 on-chip mask passes per orientation,
  all on DVE (is_equal / subtract+max chains run in 4x mode ~1.2T elem/s).

  PE uses feature-stationary matmuls with 4-way column tiling: per span,
  4 concurrent MMs with 32-col stationaries (two bases x two 32-feature
  halves) stream 512-wide mask columns -> 4 moving cols/cycle, full
  128x128 array utilization, ~216ns per span. Output is feature-major
  [f, cols] in PSUM; all bases accumulate into one bank (basis pairs
  split across partition rows 0:64 / 64:128, summed on host). A dummy
  start=True matmul with a zero moving tile owns each bank's has_written
  clear (col-tiled groups run concurrently, so no real MM may clear).

  All HBM tensors are host-pre-tiled so every DMA is one fully
  contiguous block (DMA descriptor efficiency: 1MB ~341GB/s vs 4KB-row
  slices ~220GB/s): adj ships as [ucp x vbg] blocks of [128, 2*2048]
  (uc pairs side by side, 1MB), adjT as vc-pair blocks [128, 2*1024]
  (512KB), p_stat as vc-pair blocks [128, 2*320].

  Phase 1 (out_v partial): per (vbg, uc-pair): one adj DMA + one 4-pass
  mask gen over [128, 4096]; per v-block of 512, one PSUM bank
  accumulates over all uc and bases; 4 v-block groups alternate between
  two 4-bank PSUM sets so ACT evacuation overlaps the next group's
  accumulation. Partial [16*128, 512] fp16 -> host (all-reduce over
  cores + halves-add + deg + relu).
  Phase 2 (out_u): per vc-pair: one adjT DMA + one mask gen over
  [128, 2048]; 2 PSUM banks (u-halves) accumulate over all 64 v-tiles;
  partial [2*128, 512] fp16 -> host. First vc-pairs prefetch during
  phase 1's last group to hide the transition.
"""

import numpy as np
import sys

sys.path.insert(0, "/opt/trn_rl_repo")

N_U, N_V = 8192, 8192
F = 64
R = 5
N_CORES = 8
U_SH = N_U // N_CORES          # 1024 rows per core
UC = U_SH // 128               # 8 u-chunks per core
VC = N_V // 128                # 64 v-chunks
VBW = 512                      # v-block width (phase 1 psum bank)
NVB = N_V // VBW               # 16 v-blocks
VBG = 4                        # v-block groups
HB = 4                         # v-blocks per group (psum bank set size)
W1 = N_V // VBG                # 2048 adj cols per phase-1 slice
PF2 = 3                        # phase-2 vc-pairs prefetched ahead

_CACHE = {}


def _build():
    import concourse.bass as bass
    import concourse.bacc as bacc
    import concourse.mybir as mybir
    import concourse.tile as tile

    dt = mybir.dt
    eq = mybir.AluOpType.is_equal
    add = mybir.AluOpType.add
    RELU = mybir.ActivationFunctionType.Relu

    nc = bacc.Bacc("TRN2", target_bir_lowering=False, debug=False,
                   num_devices=N_CORES)

    # adj blocks: row-block (uc*VBG+vbg) = [128, W1], each contiguous
    adj_h = nc.dram_tensor("adj_h", [UC * VBG * 128, W1],
                           dt.float16, kind="ExternalInput").ap()
    # adjT blocks: row-block jp = [128, 2*U_SH], vc pair packed
    adjt_h = nc.dram_tensor("adjt_h", [(VC // 2) * 128, 2 * U_SH],
                            dt.float16, kind="ExternalInput").ap()
    q_stat_h = nc.dram_tensor("q_stat_h", [128, UC * R * F], dt.float16,
                              kind="ExternalInput").ap()
    # p_stat blocks: row-block jp = [128, 2*R*F], vc pair packed
    p_stat_h = nc.dram_tensor("p_stat_h", [(VC // 2) * 128, 2 * R * F],
                              dt.float16, kind="ExternalInput").ap()
    out_u = nc.dram_tensor("out_u_part", [2 * 128, VBW], dt.float16,
                           kind="ExternalOutput").ap()
    out_v = nc.dram_tensor("out_v_part", [NVB * 128, VBW], dt.float16,
                           kind="ExternalOutput").ap()

    with tile.TileContext(nc) as tc:
        with tc.tile_pool(name="consts", bufs=1) as cons, \
             tc.tile_pool(name="fin", bufs=4) as fin:

            zt = cons.tile([128, VBW], dt.float16, tag="zt")
            nc.vector.memset(zt[:], 0.0)
            bias_m3 = cons.tile([128, 1], dt.float32, tag="bm3")
            nc.vector.memset(bias_m3[:], -3.0)
            # warm the ACT spline table during initial DMA wait
            warm = cons.tile([128, 1], dt.float16, tag="warm")
            nc.scalar.activation(warm[:], bias_m3[:], RELU)
            q_t = cons.tile([128, UC * R * F], dt.float16, tag="q")
            nc.scalar.dma_start(q_t[:], q_stat_h[:])

            def gen_masks(pool, src, W, key):
                """4 basis tiles from src [128, W]: M2/M3/M5 on DVE
                (is_equal), ramp4 = relu(a-3) on ACT. Accessor
                b, lo, hi -> AP, basis {0:adj, 1:M2, 2:M3, 3:r4, 4:M5}."""
                mt = pool.tile([128, 3 * W], dt.float16, tag=f"mt{key[0]}",
                               name=f"mt{key}")
                ma = pool.tile([128, W], dt.float16, tag=f"ma{key[0]}",
                               name=f"ma{key}")
                nc.vector.tensor_scalar(mt[:, 0:W], src, 2.0, None, op0=eq)
                nc.vector.tensor_scalar(mt[:, W:2 * W], src, 3.0, None,
                                        op0=eq)
                nc.scalar.activation(ma[:], src, RELU, bias=bias_m3[:, 0:1])
                nc.vector.tensor_scalar(mt[:, 2 * W:3 * W], src, 5.0, None,
                                        op0=eq)

                def basis(b, lo, hi):
                    if b == 0:
                        return src[:, lo:hi]
                    if b == 3:
                        return ma[:, lo:hi]
                    seg = {1: 0, 2: 1, 4: 2}[b]
                    return mt[:, seg * W + lo:seg * W + hi]
                return basis

            def clear_bank(ps_tile):
                nc.tensor.matmul(ps_tile[:], zt[:, 0:128], zt[:],
                                 start=True, stop=False,
                                 skip_group_check=True)

            def mm_grp(ps_tile, g, lhsT, rhs, stop=False):
                nc.tensor.matmul(ps_tile[32 * g:32 * (g + 1), :],
                                 lhsT, rhs,
                                 start=False, stop=stop,
                                 tile_position=(0, 32 * g),
                                 skip_group_check=True)

            def span(ps_tile, rhs_e, lhs_e, rhs_o, lhs_o, stop=False):
                """4-MM col-tiled span: groups 0/1 = even-basis feature
                halves vs rhs_e, groups 2/3 = odd basis vs rhs_o.
                lhs_*: (stationary tile, col offset of 64-wide block)."""
                (te, ce), (to, co) = lhs_e, lhs_o
                for g in range(4):
                    t, c0 = (te, ce) if g < 2 else (to, co)
                    rhs = rhs_e if g < 2 else rhs_o
                    fh = g % 2
                    mm_grp(ps_tile, g, t[:, c0 + 32 * fh:c0 + 32 * (fh + 1)],
                           rhs, stop=(stop and g == 3))

            # phase-2 SBUF pools opened early so the transition prefetches
            adjtp = tc.tile_pool(name="adjtp", bufs=PF2 + 1)
            adt = adjtp.__enter__()
            mpoolA = tc.tile_pool(name="maskA", bufs=3)
            mpa = mpoolA.__enter__()
            pstp = tc.tile_pool(name="pstream", bufs=PF2 + 1)
            pst = pstp.__enter__()

            p2_tiles = {}

            def p2_fetch(jp):
                if jp in p2_tiles:
                    return p2_tiles[jp]
                at = adt.tile([128, 2 * U_SH], dt.float16, tag="adjt",
                              name=f"at{jp}")
                # prefetches issued during phase 1 ride the scalar queue;
                # steady-state phase-2 fetches use sync (idle in phase 2)
                eng = nc.scalar if jp < PF2 else nc.sync
                eng.dma_start(at[:], adjt_h[jp * 128:(jp + 1) * 128, :])
                pt = pst.tile([128, 2 * R * F], dt.float16, tag="pstat",
                              name=f"pt{jp}")
                eng.dma_start(pt[:], p_stat_h[jp * 128:(jp + 1) * 128, :])
                p2_tiles[jp] = (at, pt)
                return at, pt

            # ---------------- Phase 1: out_v partial ----------------
            pspB = tc.tile_pool(name="psumB", bufs=1, space="PSUM")
            psp = pspB.__enter__()
            mpoolB = tc.tile_pool(name="maskB", bufs=6)
            mpb = mpoolB.__enter__()
            ps_v = [psp.tile([128, VBW], dt.float32, tag=f"psv{k}",
                             name=f"psv{k}") for k in range(2 * HB)]

            p1_tiles = {}

            def p1_fetch(vbg, uc):
                if (vbg, uc) in p1_tiles:
                    return p1_tiles[(vbg, uc)]
                blk = uc * VBG + vbg
                at = mpb.tile([128, W1], dt.float16,
                              tag="adj1", name=f"a{vbg}_{uc}")
                nc.sync.dma_start(
                    at[:], adj_h[blk * 128:(blk + 1) * 128, :])
                basis = gen_masks(mpb, at[:], W1, ("b", vbg, uc))
                p1_tiles[(vbg, uc)] = basis
                return basis

            seq1 = [(vbg, uc) for vbg in range(VBG) for uc in range(UC)]
            # prime the pipeline before anything else queues
            for uc in range(4):
                p1_fetch(0, uc)

            for vbg in range(VBG):
                bset = ps_v[(vbg % 2) * HB:(vbg % 2) * HB + HB]
                for vb in range(HB):
                    clear_bank(bset[vb])
                for ucp in range(UC // 2):
                    b0 = p1_fetch(vbg, 2 * ucp)
                    b1 = p1_fetch(vbg, 2 * ucp + 1)
                    k = vbg * UC + 2 * ucp + 1
                    for ka in range(k + 1, min(k + 5, len(seq1))):
                        p1_fetch(*seq1[ka])
                    last_ucp = ucp == UC // 2 - 1
                    for vb in range(HB):
                        lo, hi = vb * VBW, (vb + 1) * VBW
                        for pr in range(2):  # pairs (adj,M2), (M3,r4)
                            for i, bs in ((0, b0), (1, b1)):
                                qc = (2 * ucp + i) * R * F
                                span(bset[vb],
                                     bs(2 * pr, lo, hi),
                                     (q_t, qc + (2 * pr) * F),
                                     bs(2 * pr + 1, lo, hi),
                                     (q_t, qc + (2 * pr + 1) * F))
                        # pair3: M5 of uc-even on grp 0/1, uc-odd on 2/3
                        span(bset[vb],
                             b0(4, lo, hi),
                             (q_t, (2 * ucp) * R * F + 4 * F),
                             b1(4, lo, hi),
                             (q_t, (2 * ucp + 1) * R * F + 4 * F),
                             stop=last_ucp)
                if vbg == VBG - 1:
                    for jp in range(PF2):
                        p2_fetch(jp)
                # evacuate this vbg's banks (overlaps next vbg's spans)
                for vb in range(HB):
                    ev = fin.tile([128, VBW], dt.float16, tag="evacv",
                                  name=f"evv{vbg}_{vb}")
                    if vb % 2 == 0:
                        nc.vector.tensor_scalar(ev[:], bset[vb][:], 0.0,
                                                None, op0=add)
                    else:
                        nc.scalar.copy(ev[:], bset[vb][:])
                    nvb = vbg * HB + vb
                    nc.gpsimd.dma_start(
                        out_v[nvb * 128:(nvb + 1) * 128, :], ev[:])
            mpoolB.__exit__(None, None, None)
            pspB.__exit__(None, None, None)

            # ---------------- Phase 2: out_u partial ----------------
            pspA = tc.tile_pool(name="psumA", bufs=1, space="PSUM")
            pspa = pspA.__enter__()
            ps_u = [pspa.tile([128, VBW], dt.float32, tag=f"psu{h}",
                              name=f"psu{h}") for h in range(2)]
            for h in range(2):
                clear_bank(ps_u[h])
            p2_basis = {}

            def p2_gen(jp):
                if jp not in p2_basis:
                    at, _ = p2_fetch(jp)
                    p2_basis[jp] = gen_masks(mpa, at[:], 2 * U_SH,
                                             ("p", jp))
                return p2_basis[jp]

            for jp in range(VC // 2):
                _, pt = p2_fetch(jp)
                basis = p2_gen(jp)
                if jp + 1 < VC // 2:
                    p2_gen(jp + 1)
                if jp + PF2 < VC // 2:
                    p2_fetch(jp + PF2)
                for i in range(2):
                    vc = 2 * jp + i
                    last = vc == VC - 1
                    ioff = i * U_SH
                    poff = i * R * F
                    for h in range(2):
                        lo, hi = ioff + h * VBW, ioff + (h + 1) * VBW
                        for pr in range(2):
                            span(ps_u[h],
                                 basis(2 * pr, lo, hi),
                                 (pt, poff + (2 * pr) * F),
                                 basis(2 * pr + 1, lo, hi),
                                 (pt, poff + (2 * pr + 1) * F))
                    # pair3: M5 u-half0 on grp 0/1 (bank 0 rows 0:64),
                    #        M5 u-half1 on grp 2/3 (bank 1 rows 64:128)
                    for g in range(4):
                        h = g // 2
                        fh = g % 2
                        mm_grp(ps_u[h], g,
                               pt[:, poff + 4 * F + 32 * fh:
                                  poff + 4 * F + 32 * (fh + 1)],
                               basis(4, ioff + h * VBW, ioff + (h + 1) * VBW),
                               stop=(last and fh == 1))
            for h in range(2):
                ev = fin.tile([128, VBW], dt.float16, tag="evacu",
                              name=f"evu{h}")
                nc.vector.tensor_scalar(ev[:], ps_u[h][:], 0.0, None,
                                        op0=add)
                nc.gpsimd.dma_start(out_u[h * 128:(h + 1) * 128, :], ev[:])
            pstp.__exit__(None, None, None)
            mpoolA.__exit__(None, None, None)
            adjtp.__exit__(None, None, None)
            pspA.__exit__(None, None, None)

    nc.compile()
    return nc


def _host_prep(adj, u_feature, v_feature, weight_u, weight_v):
    adj = np.asarray(adj)
    u_feature = np.asarray(u_feature, dtype=np.float32)
    v_feature = np.asarray(v_feature, dtype=np.float32)
    weight_u = np.asarray(weight_u, dtype=np.float32)
    weight_v = np.asarray(weight_v, dtype=np.float32)

    adj16 = adj.astype(np.float16)

    def basis_fold(X):
        """X [R, n, F] per-rating projections -> basis projections for
        {adj, M2, M3, ramp4, M5}: sum_r M_r X_r = a*X1 + M2*(X2-2X1)
        + M3*(X3-3X1) + r4*(X4-4X1) + M5*(X5+3X1-2X4)   (r4(5)=2)."""
        B = np.empty_like(X)
        B[0] = X[0]
        B[1] = X[1] - 2 * X[0]
        B[2] = X[2] - 3 * X[0]
        B[3] = X[3] - 4 * X[0]
        B[4] = X[4] + 3 * X[0] - 2 * X[3]
        return B

    # P_b = v_feat @ W_u basis-folded  [R, N_V, F]
    P = basis_fold(np.einsum("vf,rfo->rvo", v_feature, weight_u))
    ps = P.reshape(R, VC, 128, F).transpose(1, 2, 0, 3).reshape(VC, 128,
                                                                R * F)
    p_stat = np.concatenate([ps[0::2], ps[1::2]], axis=2).reshape(
        (VC // 2) * 128, 2 * R * F).astype(np.float16)
    p_stat = np.ascontiguousarray(p_stat)

    in_maps = []
    for c in range(N_CORES):
        sl = slice(c * U_SH, (c + 1) * U_SH)
        Q = basis_fold(np.einsum("uf,rfo->ruo", u_feature[sl], weight_v))
        q_stat = np.ascontiguousarray(
            Q.reshape(R, UC, 128, F).transpose(2, 1, 0, 3).reshape(128, -1)
        ).astype(np.float16)
        a = adj16[sl]
        # adj blocks [uc*VBG+vbg] = [128, W1], contiguous per block
        adj_b = a.reshape(UC, 128, VBG, W1).transpose(0, 2, 1, 3).reshape(
            UC * VBG * 128, W1)
        # adjT blocks jp = [128, vc-even 1024 | vc-odd 1024]
        T = np.ascontiguousarray(a.T).reshape(VC, 128, U_SH)
        adjt_b = np.concatenate([T[0::2], T[1::2]], axis=2).reshape(
            (VC // 2) * 128, 2 * U_SH)
        in_maps.append({
            "adj_h": np.ascontiguousarray(adj_b),
            "adjt_h": np.ascontiguousarray(adjt_b),
            "q_stat_h": q_stat,
            "p_stat_h": p_stat,
        })
    return in_maps


def kernel(adj, u_feature, v_feature, weight_u, weight_v, _trace=False):
    from concourse import bass_utils

    if "nc" not in _CACHE:
        _CACHE["nc"] = _build()
    nc = _CACHE["nc"]

    in_maps = _host_prep(adj, u_feature, v_feature, weight_u, weight_v)
    res = bass_utils.run_bass_kernel_spmd(
        nc, in_maps, core_ids=list(range(N_CORES)), trace=_trace)
    _CACHE["last_result"] = res

    adj = np.asarray(adj)
    deg_u = (adj > 0).sum(axis=1).astype(np.float64)
    deg_v = (adj > 0).sum(axis=0).astype(np.float64)
    d_u = np.where(deg_u > 0, 1.0 / np.maximum(deg_u, 0.5), 0.0)
    d_v = np.where(deg_v > 0, 1.0 / np.maximum(deg_v, 0.5), 0.0)

    # out_u partial per core: [256, 512]: rows h*128.. = u-half h, row
    # layout [f0:32 even bases | f32:64 even | f0:32 odd | f32:64 odd]
    outs = []
    for c in range(N_CORES):
        p = res.results[c]["out_u_part"].astype(np.float32)
        x = np.concatenate([p[0:128], p[128:256]], axis=1)   # [128, 1024]
        ut = (x[0:32] + x[64:96], x[32:64] + x[96:128])
        outs.append(np.concatenate(ut, axis=0))              # [64, 1024]
    out_uT = np.concatenate(outs, axis=1)                    # [64, 8192]
    out_u = np.maximum(out_uT.T * d_u[:, None], 0.0).astype(np.float32)

    acc = np.zeros((128, N_V), np.float64)
    for c in range(N_CORES):
        p = res.results[c]["out_v_part"].astype(np.float64)
        acc += np.concatenate(
            [p[k * 128:(k + 1) * 128] for k in range(NVB)], axis=1)
    out_vT = np.concatenate(
        [acc[0:32] + acc[64:96], acc[32:64] + acc[96:128]], axis=0)
    out_v = np.maximum(out_vT.T * d_v[:, None], 0.0).astype(np.float32)
    return out_u, out_v


# revision 43
# speedup vs baseline: 1.0346x; 1.0346x over previous
"""Bipartite graph convolution (GCMC-style) Trainium2 kernel, 8-core SPMD.

Math (reference): per-rating masks M_r = (adj == r), r=1..5,
  out_u = relu(d_u * sum_r (M_r @ v_feat) @ W_u[r]),  d_u = 1/deg_u
  out_v = relu(d_v * sum_r (M_r.T @ u_feat) @ W_v[r]), d_v = 1/deg_v

Device formulation (per core, u-rows sharded 1024/core), v4:
  Fold weights on host: P_b = v_feat @ W~_u[b]  [8192, 64] per basis b,
  Q_b = u_shard @ W~_v[b] [1024, 64], basis {adj, M2, M3, ramp4, M5}
  (adj = sum_r r*M_r) so only 4 on-chip mask passes per orientation,
  all on DVE (is_equal / subtract+max chains run in 4x mode ~1.2T elem/s).

  PE uses feature-stationary matmuls with 4-way column tiling: per span,
  4 concurrent MMs with 32-col stationaries (two bases x two 32-feature
  halves) stream 512-wide mask columns -> 4 moving cols/cycle, full
  128x128 array utilization, ~216ns per span. Output is feature-major
  [f, cols] in PSUM; all bases accumulate into one bank (basis pairs
  split across partition rows 0:64 / 64:128, summed on host). A dummy
  start=True matmul with a zero moving tile owns each bank's has_written
  clear (col-tiled groups run concurrently, so no real MM may clear).

  All HBM tensors are host-pre-tiled so every DMA is one fully
  contiguous block (DMA descriptor efficiency: 1MB ~341GB/s vs 4KB-row
  slices ~220GB/s): adj ships as [ucp x vbg] blocks of [128, 2*2048]
  (uc pairs side by side, 1MB), adjT as vc-pair blocks [128, 2*1024]
  (512KB), p_stat as vc-pair blocks [128, 2*320].

  Phase 1 (out_v partial): per (vbg, uc-pair): one adj DMA + one 4-pass
  mask gen over [128, 4096]; per v-block of 512, one PSUM bank
  accumulates over all uc and bases; 4 v-block groups alternate between
  two 4-bank PSUM sets so ACT evacuation overlaps the next group's
  accumulation. Partial [16*128, 512] fp16 -> host (all-reduce over
  cores + halves-add + deg + relu).
  Phase 2 (out_u): per vc-pair: one adjT DMA + one mask gen over
  [128, 2048]; 2 PSUM banks (u-halves) accumulate over all 64 v-tiles;
  partial [2*128, 512] fp16 -> host. First vc-pairs prefetch during
  phase 1's last group to hide the transition.
"""

import numpy as np
import sys

sys.path.insert(0, "/opt/trn_rl_repo")

N_U, N_V = 8192, 8192
F = 64
R = 5
N_CORES = 8
U_SH = N_U // N_CORES          # 1024 rows per core
UC = U_SH // 128               # 8 u-chunks per core
VC = N_V // 128                # 64 v-chunks
VBW = 512                      # v-block width (phase 1 psum bank)
NVB = N_V // VBW               # 16 v-blocks
VBG = 4                        # v-block groups
HB = 4                         # v-blocks per group (psum bank set size)
W1 = N_V // VBG                # 2048 adj cols per phase-1 slice
PF2 = 3                        # phase-2 vc-pairs prefetched ahead

_CACHE = {}


def _build():
    import concourse.bass as bass
    import concourse.bacc as bacc
    import concourse.mybir as mybir
    import concourse.tile as tile

    dt = mybir.dt
    eq = mybir.AluOpType.is_equal
    add = mybir.AluOpType.add
    RELU = mybir.ActivationFunctionType.Relu

    nc = bacc.Bacc("TRN2", target_bir_lowering=False, debug=False,
                   num_devices=N_CORES)

    # adj blocks: row-block (ucp*VBG+vbg) = [128, 2*W1], uc pair packed
    adj_h = nc.dram_tensor("adj_h", [(UC // 2) * VBG * 128, 2 * W1],
                           dt.float16, kind="ExternalInput").ap()
    # adjT blocks: row-block jp = [128, 2*U_SH], vc pair packed
    adjt_h = nc.dram_tensor("adjt_h", [(VC // 2) * 128, 2 * U_SH],
                            dt.float16, kind="ExternalInput").ap()
    q_stat_h = nc.dram_tensor("q_stat_h", [128, UC * R * F], dt.float16,
                              kind="ExternalInput").ap()
    # p_stat blocks: row-block jp = [128, 2*R*F], vc pair packed
    p_stat_h = nc.dram_tensor("p_stat_h", [(VC // 2) * 128, 2 * R * F],
                              dt.float16, kind="ExternalInput").ap()
    out_u = nc.dram_tensor("out_u_part", [2 * 128, VBW], dt.float16,
                           kind="ExternalOutput").ap()
    out_v = nc.dram_tensor("out_v_part", [NVB * 128, VBW], dt.float16,
                           kind="ExternalOutput").ap()

    with tile.TileContext(nc) as tc:
        with tc.tile_pool(name="consts", bufs=1) as cons, \
             tc.tile_pool(name="fin", bufs=4) as fin:

            zt = cons.tile([128, VBW], dt.float16, tag="zt")
            nc.vector.memset(zt[:], 0.0)
            bias_m3 = cons.tile([128, 1], dt.float32, tag="bm3")
            nc.vector.memset(bias_m3[:], -3.0)
            # warm the ACT spline table during initial DMA wait
            warm = cons.tile([128, 1], dt.float16, tag="warm")
            nc.scalar.activation(warm[:], bias_m3[:], RELU)
            q_t = cons.tile([128, UC * R * F], dt.float16, tag="q")
            nc.scalar.dma_start(q_t[:], q_stat_h[:])

            def gen_masks(pool, src, W, key):
                """4 basis tiles from src [128, W]: M2/M3/M5 on DVE
                (is_equal), ramp4 = relu(a-3) on ACT. Accessor
                b, lo, hi -> AP, basis {0:adj, 1:M2, 2:M3, 3:r4, 4:M5}."""
                mt = pool.tile([128, 3 * W], dt.float16, tag=f"mt{key[0]}",
                               name=f"mt{key}")
                ma = pool.tile([128, W], dt.float16, tag=f"ma{key[0]}",
                               name=f"ma{key}")
                nc.vector.tensor_scalar(mt[:, 0:W], src, 2.0, None, op0=eq)
                nc.vector.tensor_scalar(mt[:, W:2 * W], src, 3.0, None,
                                        op0=eq)
                nc.scalar.activation(ma[:], src, RELU, bias=bias_m3[:, 0:1])
                nc.vector.tensor_scalar(mt[:, 2 * W:3 * W], src, 5.0, None,
                                        op0=eq)

                def basis(b, lo, hi):
                    if b == 0:
                        return src[:, lo:hi]
                    if b == 3:
                        return ma[:, lo:hi]
                    seg = {1: 0, 2: 1, 4: 2}[b]
                    return mt[:, seg * W + lo:seg * W + hi]
                return basis

            def clear_bank(ps_tile):
                nc.tensor.matmul(ps_tile[:], zt[:, 0:128], zt[:],
                                 start=True, stop=False,
                                 skip_group_check=True)

            def mm_grp(ps_tile, g, lhsT, rhs, stop=False):
                nc.tensor.matmul(ps_tile[32 * g:32 * (g + 1), :],
                                 lhsT, rhs,
                                 start=False, stop=stop,
                                 tile_position=(0, 32 * g),
                                 skip_group_check=True)

            def span(ps_tile, rhs_e, lhs_e, rhs_o, lhs_o, stop=False):
                """4-MM col-tiled span: groups 0/1 = even-basis feature
                halves vs rhs_e, groups 2/3 = odd basis vs rhs_o.
                lhs_*: (stationary tile, col offset of 64-wide block)."""
                (te, ce), (to, co) = lhs_e, lhs_o
                for g in range(4):
                    t, c0 = (te, ce) if g < 2 else (to, co)
                    rhs = rhs_e if g < 2 else rhs_o
                    fh = g % 2
                    mm_grp(ps_tile, g, t[:, c0 + 32 * fh:c0 + 32 * (fh + 1)],
                           rhs, stop=(stop and g == 3))

            # phase-2 SBUF pools opened early so the transition prefetches
            adjtp = tc.tile_pool(name="adjtp", bufs=PF2 + 1)
            adt = adjtp.__enter__()
            mpoolA = tc.tile_pool(name="maskA", bufs=3)
            mpa = mpoolA.__enter__()
            pstp = tc.tile_pool(name="pstream", bufs=PF2 + 1)
            pst = pstp.__enter__()

            p2_tiles = {}

            def p2_fetch(jp):
                if jp in p2_tiles:
                    return p2_tiles[jp]
                at = adt.tile([128, 2 * U_SH], dt.float16, tag="adjt",
                              name=f"at{jp}")
                # prefetches issued during phase 1 ride the scalar queue;
                # steady-state phase-2 fetches use sync (idle in phase 2)
                eng = nc.scalar if jp < PF2 else nc.sync
                eng.dma_start(at[:], adjt_h[jp * 128:(jp + 1) * 128, :])
                pt = pst.tile([128, 2 * R * F], dt.float16, tag="pstat",
                              name=f"pt{jp}")
                eng.dma_start(pt[:], p_stat_h[jp * 128:(jp + 1) * 128, :])
                p2_tiles[jp] = (at, pt)
                return at, pt

            # ---------------- Phase 1: out_v partial ----------------
            pspB = tc.tile_pool(name="psumB", bufs=1, space="PSUM")
            psp = pspB.__enter__()
            mpoolB = tc.tile_pool(name="maskB", bufs=3)
            mpb = mpoolB.__enter__()
            ps_v = [psp.tile([128, VBW], dt.float32, tag=f"psv{k}",
                             name=f"psv{k}") for k in range(2 * HB)]

            p1_tiles = {}

            def p1_fetch(vbg, ucp):
                if (vbg, ucp) in p1_tiles:
                    return p1_tiles[(vbg, ucp)]
                blk = ucp * VBG + vbg
                at = mpb.tile([128, 2 * W1], dt.float16,
                              tag="adj1", name=f"a{vbg}_{ucp}")
                nc.sync.dma_start(
                    at[:], adj_h[blk * 128:(blk + 1) * 128, :])
                basis = gen_masks(mpb, at[:], 2 * W1, ("b", vbg, ucp))
                p1_tiles[(vbg, ucp)] = basis
                return basis

            seq1 = [(vbg, ucp) for vbg in range(VBG)
                    for ucp in range(UC // 2)]
            # prime the pipeline before anything else queues
            p1_fetch(0, 0)
            p1_fetch(0, 1)

            for vbg in range(VBG):
                bset = ps_v[(vbg % 2) * HB:(vbg % 2) * HB + HB]
                for vb in range(HB):
                    clear_bank(bset[vb])
                for ucp in range(UC // 2):
                    basis = p1_fetch(vbg, ucp)
                    k = vbg * (UC // 2) + ucp
                    for ka in (k + 1, k + 2):
                        if ka < len(seq1):
                            p1_fetch(*seq1[ka])
                    last_ucp = ucp == UC // 2 - 1
                    for vb in range(HB):
                        for pr in range(2):  # pairs (adj,M2), (M3,r4)
                            for i in range(2):
                                lo = i * W1 + vb * VBW
                                hi = lo + VBW
                                qc = (2 * ucp + i) * R * F
                                span(bset[vb],
                                     basis(2 * pr, lo, hi),
                                     (q_t, qc + (2 * pr) * F),
                                     basis(2 * pr + 1, lo, hi),
                                     (q_t, qc + (2 * pr + 1) * F))
                        # pair3: M5 of uc-even on grp 0/1, uc-odd on 2/3
                        lo0, hi0 = vb * VBW, vb * VBW + VBW
                        lo1, hi1 = W1 + vb * VBW, W1 + vb * VBW + VBW
                        span(bset[vb],
                             basis(4, lo0, hi0),
                             (q_t, (2 * ucp) * R * F + 4 * F),
                             basis(4, lo1, hi1),
                             (q_t, (2 * ucp + 1) * R * F + 4 * F),
                             stop=last_ucp)
                if vbg == VBG - 1:
                    for jp in range(PF2):
                        p2_fetch(jp)
                # evacuate this vbg's banks (overlaps next vbg's spans)
                for vb in range(HB):
                    ev = fin.tile([128, VBW], dt.float16, tag="evacv",
                                  name=f"evv{vbg}_{vb}")
                    if vb % 2 == 0:
                        nc.vector.tensor_scalar(ev[:], bset[vb][:], 0.0,
                                                None, op0=add)
                    else:
                        nc.scalar.copy(ev[:], bset[vb][:])
                    nvb = vbg * HB + vb
                    nc.gpsimd.dma_start(
                        out_v[nvb * 128:(nvb + 1) * 128, :], ev[:])
            mpoolB.__exit__(None, None, None)
            pspB.__exit__(None, None, None)

            # ---------------- Phase 2: out_u partial ----------------
            pspA = tc.tile_pool(name="psumA", bufs=1, space="PSUM")
            pspa = pspA.__enter__()
            ps_u = [pspa.tile([128, VBW], dt.float32, tag=f"psu{h}",
                              name=f"psu{h}") for h in range(2)]
            for h in range(2):
                clear_bank(ps_u[h])
            p2_basis = {}

            def p2_gen(jp):
                if jp not in p2_basis:
                    at, _ = p2_fetch(jp)
                    p2_basis[jp] = gen_masks(mpa, at[:], 2 * U_SH,
                                             ("p", jp))
                return p2_basis[jp]

            for jp in range(VC // 2):
                _, pt = p2_fetch(jp)
                basis = p2_gen(jp)
                if jp + 1 < VC // 2:
                    p2_gen(jp + 1)
                if jp + PF2 < VC // 2:
                    p2_fetch(jp + PF2)
                for i in range(2):
                    vc = 2 * jp + i
                    last = vc == VC - 1
                    ioff = i * U_SH
                    poff = i * R * F
                    for h in range(2):
                        lo, hi = ioff + h * VBW, ioff + (h + 1) * VBW
                        for pr in range(2):
                            span(ps_u[h],
                                 basis(2 * pr, lo, hi),
                                 (pt, poff + (2 * pr) * F),
                                 basis(2 * pr + 1, lo, hi),
                                 (pt, poff + (2 * pr + 1) * F))
                    # pair3: M5 u-half0 on grp 0/1 (bank 0 rows 0:64),
                    #        M5 u-half1 on grp 2/3 (bank 1 rows 64:128)
                    for g in range(4):
                        h = g // 2
                        fh = g % 2
                        mm_grp(ps_u[h], g,
                               pt[:, poff + 4 * F + 32 * fh:
                                  poff + 4 * F + 32 * (fh + 1)],
                               basis(4, ioff + h * VBW, ioff + (h + 1) * VBW),
                               stop=(last and fh == 1))
            for h in range(2):
                ev = fin.tile([128, VBW], dt.float16, tag="evacu",
                              name=f"evu{h}")
                nc.vector.tensor_scalar(ev[:], ps_u[h][:], 0.0, None,
                                        op0=add)
                nc.gpsimd.dma_start(out_u[h * 128:(h + 1) * 128, :], ev[:])
            pstp.__exit__(None, None, None)
            mpoolA.__exit__(None, None, None)
            adjtp.__exit__(None, None, None)
            pspA.__exit__(None, None, None)

    nc.compile()
    return nc


def _host_prep(adj, u_feature, v_feature, weight_u, weight_v):
    adj = np.asarray(adj)
    u_feature = np.asarray(u_feature, dtype=np.float32)
    v_feature = np.asarray(v_feature, dtype=np.float32)
    weight_u = np.asarray(weight_u, dtype=np.float32)
    weight_v = np.asarray(weight_v, dtype=np.float32)

    adj16 = adj.astype(np.float16)

    def basis_fold(X):
        """X [R, n, F] per-rating projections -> basis projections for
        {adj, M2, M3, ramp4, M5}: sum_r M_r X_r = a*X1 + M2*(X2-2X1)
        + M3*(X3-3X1) + r4*(X4-4X1) + M5*(X5+3X1-2X4)   (r4(5)=2)."""
        B = np.empty_like(X)
        B[0] = X[0]
        B[1] = X[1] - 2 * X[0]
        B[2] = X[2] - 3 * X[0]
        B[3] = X[3] - 4 * X[0]
        B[4] = X[4] + 3 * X[0] - 2 * X[3]
        return B

    # P_b = v_feat @ W_u basis-folded  [R, N_V, F]
    P = basis_fold(np.einsum("vf,rfo->rvo", v_feature, weight_u))
    ps = P.reshape(R, VC, 128, F).transpose(1, 2, 0, 3).reshape(VC, 128,
                                                                R * F)
    p_stat = np.concatenate([ps[0::2], ps[1::2]], axis=2).reshape(
        (VC // 2) * 128, 2 * R * F).astype(np.float16)
    p_stat = np.ascontiguousarray(p_stat)

    in_maps = []
    for c in range(N_CORES):
        sl = slice(c * U_SH, (c + 1) * U_SH)
        Q = basis_fold(np.einsum("uf,rfo->ruo", u_feature[sl], weight_v))
        q_stat = np.ascontiguousarray(
            Q.reshape(R, UC, 128, F).transpose(2, 1, 0, 3).reshape(128, -1)
        ).astype(np.float16)
        a = adj16[sl]
        # adj blocks [ucp*VBG+vbg] = [128, uc-even W1 | uc-odd W1]
        A = a.reshape(UC, 128, VBG, W1).transpose(0, 2, 1, 3)  # [UC,VBG,128,W1]
        adj_b = np.concatenate([A[0::2], A[1::2]], axis=3).reshape(
            (UC // 2) * VBG * 128, 2 * W1)
        # adjT blocks jp = [128, vc-even 1024 | vc-odd 1024]
        T = np.ascontiguousarray(a.T).reshape(VC, 128, U_SH)
        adjt_b = np.concatenate([T[0::2], T[1::2]], axis=2).reshape(
            (VC // 2) * 128, 2 * U_SH)
        in_maps.append({
            "adj_h": np.ascontiguousarray(adj_b),
            "adjt_h": np.ascontiguousarray(adjt_b),
            "q_stat_h": q_stat,
            "p_stat_h": p_stat,
        })
    return in_maps


def kernel(adj, u_feature, v_feature, weight_u, weight_v, _trace=False):
    from concourse import bass_utils

    if "nc" not in _CACHE:
        _CACHE["nc"] = _build()
    nc = _CACHE["nc"]

    in_maps = _host_prep(adj, u_feature, v_feature, weight_u, weight_v)
    res = bass_utils.run_bass_kernel_spmd(
        nc, in_maps, core_ids=list(range(N_CORES)), trace=_trace)
    _CACHE["last_result"] = res

    adj = np.asarray(adj)
    deg_u = (adj > 0).sum(axis=1).astype(np.float64)
    deg_v = (adj > 0).sum(axis=0).astype(np.float64)
    d_u = np.where(deg_u > 0, 1.0 / np.maximum(deg_u, 0.5), 0.0)
    d_v = np.where(deg_v > 0, 1.0 / np.maximum(deg_v, 0.5), 0.0)

    # out_u partial per core: [256, 512]: rows h*128.. = u-half h, row
    # layout [f0:32 even bases | f32:64 even | f0:32 odd | f32:64 odd]
    outs = []
    for c in range(N_CORES):
        p = res.results[c]["out_u_part"].astype(np.float32)
        x = np.concatenate([p[0:128], p[128:256]], axis=1)   # [128, 1024]
        ut = (x[0:32] + x[64:96], x[32:64] + x[96:128])
        outs.append(np.concatenate(ut, axis=0))              # [64, 1024]
    out_uT = np.concatenate(outs, axis=1)                    # [64, 8192]
    out_u = np.maximum(out_uT.T * d_u[:, None], 0.0).astype(np.float32)

    acc = np.zeros((128, N_V), np.float64)
    for c in range(N_CORES):
        p = res.results[c]["out_v_part"].astype(np.float64)
        acc += np.concatenate(
            [p[k * 128:(k + 1) * 128] for k in range(NVB)], axis=1)
    out_vT = np.concatenate(
        [acc[0:32] + acc[64:96], acc[32:64] + acc[96:128]], axis=0)
    out_v = np.maximum(out_vT.T * d_v[:, None], 0.0).astype(np.float32)
    return out_u, out_v
